# revision 20
# baseline (speedup 1.0000x reference)
"""Dense MoE (BasicMoE) Trainium2 Bass kernel.

Problem (hardcoded): x [4, 2048, 1024] f32, gate_w [1024, 8], gate_b [8],
expert_w [8, 1024, 1024], expert_b [8, 1024].

    tok = x.reshape(T, H)
    w   = softmax(tok @ gate_w + gate_b)           # [T, E]
    eo  = einsum('th,ehd->ted', tok, expert_w) + expert_b
    out = einsum('te,ted->td', w, eo)              # [T, H]

Sharding: tokens split across 8 cores (data parallel), weights replicated.

Per-core algorithm (T_l = 1024 tokens). The TensorEngine contracts along
the partition dim, so the activations are needed h-major (xT); that
transpose is pure data layout, done host-side when sharding.

  1. Gate, in transposed [e, t] layout: logitsT = gate_w.T @ x.T via
     matmuls with gate_w slices stationary (8-wide LDWEIGHTS);
     ewT = exp(logitsT + gate_b) with gate_b as a natural per-partition
     ACT bias. Small PE transposes give ew in [t, e] layout, where
     S = sum_e ew and ews = ew * (1/S) are per-partition ops. softmax's
     division is linear in the combine, so it is folded into the gate
     weights and nothing needs normalizing at the end. ews is transposed
     back (ewsT) for the bias term.
  2. acc[t,d] seeded with the bias term sum_e ews[t,e]*b_e[d] (K=8 matmul
     of ewsT against expert_b).
  3. For each expert: y_e = xT.T @ W_e accumulated over k in PSUM
     (bf16 operands, fp32 accumulation, full PE rate with fast weight
     load), evicted scaled by ews[:,e] (per-partition scale, alternating
     ACT/DVE) and added into an SBUF accumulator by DVE.
  4. acc IS the output: DMA out per 512-wide half as soon as the last
     expert's contribution lands.
"""

import os
from contextlib import ExitStack

import numpy as np

import concourse.tile as tile
from concourse import bacc, mybir
from concourse.bass_utils import run_bass_kernel_spmd
from concourse.masks import make_identity

B, S, H, E = 4, 2048, 1024, 8
T = B * S
N_CORES = 8
TL = T // N_CORES          # tokens per core = 1024
P = 128                    # SBUF partitions
KT = H // P                # 8 contraction tiles
MT = TL // P               # 8 token tiles per core
DH = 512                   # matmul moving free-dim (fp32 PSUM bank)
ND = H // DH               # 2 d-halves
XC = 2                     # x DMA column chunks (queue parallelism)
OC = 2                     # output DMA column chunks per (m, half)

F32 = mybir.dt.float32
F32R = mybir.dt.float32r
BF16 = mybir.dt.bfloat16

_CACHE = {}
LAST_RESULT = None


def _r(ap):
    """Bitcast an f32 AP to float32r (same bits; PE rounds internally)."""
    return ap.bitcast(F32R)


def _build_moe_nc():
    nc = bacc.Bacc(
        "TRN2",
        target_bir_lowering=False,
        debug=False,
        enable_asserts=False,
        num_devices=N_CORES,
    )

    x_shT = nc.dram_tensor("x_shT", [H, TL], BF16, kind="ExternalInput").ap()
    gate_w = nc.dram_tensor("gate_w", [H, E], BF16, kind="ExternalInput").ap()
    gate_b = nc.dram_tensor("gate_b", [E], F32, kind="ExternalInput").ap()
    expert_w = nc.dram_tensor("expert_w", [E, H, H], BF16, kind="ExternalInput").ap()
    expert_b = nc.dram_tensor("expert_b", [E, H], F32, kind="ExternalInput").ap()
    out_sh = nc.dram_tensor("out_sh", [TL, H], F32, kind="ExternalOutput").ap()

    with tile.TileContext(nc) as tc, ExitStack() as ctx:
        const = ctx.enter_context(tc.tile_pool(name="const", bufs=1))
        wpool = ctx.enter_context(tc.tile_pool(name="wpool", bufs=2))
        accp = ctx.enter_context(tc.tile_pool(name="accp", bufs=1))
        tmp = ctx.enter_context(tc.tile_pool(name="tmp", bufs=6))
        psum_s = tc.alloc_tile_pool(name="psum_s", bufs=1, space="PSUM")

        ident = const.tile([P, P], F32)
        make_identity(nc, ident)

        ident_bf = const.tile([E, E], BF16)
        make_identity(nc, ident_bf)

        # ---- loads ------------------------------------------------------
        gw = const.tile([P, KT, E], BF16)
        nc.sync.dma_start(gw, gate_w.rearrange("(k p) e -> p k e", p=P))
        gb8 = const.tile([E, 1], F32)
        nc.sync.dma_start(gb8, gate_b[:, None])
        eb = const.tile([E, H], F32R)
        nc.sync.dma_start(eb, _r(expert_b))

        # xT: h on partitions, t on free — straight (contiguous) DMA from the
        # host-transposed shard. GpSimd SWDGE queues, half-column chunks in
        # half-major order so the first gate matmul's operands land first.
        xT = [const.tile([P, TL], BF16, name=f"xT{k}") for k in range(KT)]
        xcw = TL // XC
        for c in range(XC):
            for k in range(KT):
                csl = slice(c * xcw, (c + 1) * xcw)
                eng = nc.gpsimd if k % 2 == 0 else nc.sync
                eng.dma_start(
                    xT[k][:, csl], x_shT[k * P : (k + 1) * P, csl]
                )

        # ---- gate -------------------------------------------------------
        ewT_raw = const.tile([E, TL], BF16)   # exp(logits).T (unnormalized)
        ews = const.tile([P, MT, E], F32)     # per-token gate weight / S
        ewsT = const.tile([E, TL], F32R)      # ews transposed, for bias mm

        for h2 in range(2):
            hsl = slice(h2 * DH, (h2 + 1) * DH)
            pgT = psum_s.tile([E, DH], F32, tag="sm", bufs=2)
            for k in range(KT):
                nc.tensor.matmul(
                    pgT,
                    lhsT=gw[:, k, :],
                    rhs=xT[k][:, hsl],
                    start=(k == 0),
                    stop=(k == KT - 1),
                )
            # ewT = exp(logitsT + gate_b); gate_b is per-partition here
            nc.scalar.activation(
                ewT_raw[:, hsl], pgT, mybir.ActivationFunctionType.Exp, bias=gb8
            )

        for m in range(MT):
            msl = slice(m * P, (m + 1) * P)
            # ew[t, e] for this token tile via PE transpose
            ptw = psum_s.tile([P, E], BF16, tag="sm", bufs=2)
            nc.tensor.transpose(ptw, ewT_raw[:, msl], ident_bf)
            ssum = tmp.tile([P, 1], F32, tag="ssum")
            nc.vector.reduce_sum(ssum, ptw, axis=mybir.AxisListType.X)
            inv = tmp.tile([P, 1], F32, tag="inv")
            nc.vector.reciprocal(inv, ssum)
            nc.vector.tensor_scalar_mul(ews[:, m, :], ptw, inv)
            # back-transpose the normalized weights for the bias matmul
            ptb = psum_s.tile([E, P], F32, tag="sm", bufs=2)
            nc.tensor.transpose(ptb, ews[:, m, :], ident)
            nc.vector.tensor_copy(ewsT[:, msl], _r(ptb))

        # gate done: release its psum banks, give the expert loop all 8
        psum_s.release()
        psum = ctx.enter_context(tc.tile_pool(name="psum", bufs=8, space="PSUM"))

        # ---- bias seed: acc = ews @ expert_b ---------------------------
        acc = [accp.tile([P, H], F32, name=f"acc{m}") for m in range(MT)]
        for m in range(MT):
            msl = slice(m * P, (m + 1) * P)
            for n in range(ND):
                nsl = slice(n * DH, (n + 1) * DH)
                pb = psum.tile([P, DH], F32, tag="ps")
                nc.tensor.matmul(
                    pb, lhsT=ewsT[:, msl], rhs=eb[:, nsl], start=True, stop=True
                )
                nc.vector.tensor_copy(acc[m][:, nsl], pb)

        # ---- experts ----------------------------------------------------
        ocw = DH // OC
        for e in range(E):
            wsb = wpool.tile([P, KT, H], BF16, tag="w")
            # e=0 is latency-critical (PE is waiting): split across both
            # HWDGE and SWDGE queue sets in small chunks. Steady state uses
            # the sync queues only.
            ewc = 4 if e == 0 else (2 if e == 1 else 1)
            wcw = H // ewc
            for c in range(ewc):
                for k in range(KT):
                    csl = slice(c * wcw, (c + 1) * wcw)
                    eng = nc.gpsimd if (e < 2 and k % 2 == 1) else nc.sync
                    eng.dma_start(
                        wsb[:, k, csl],
                        expert_w[e, k * P : (k + 1) * P, csl],
                    )
            last = e == E - 1
            for n in range(ND):
                nsl = slice(n * DH, (n + 1) * DH)
                for m in range(MT):
                    msl = slice(m * P, (m + 1) * P)
                    ps = psum.tile([P, DH], F32, tag="ps")
                    for k in range(KT):
                        nc.tensor.matmul(
                            ps,
                            lhsT=xT[k][:, msl],
                            rhs=wsb[:, k, nsl],
                            start=(k == 0),
                            stop=(k == KT - 1),
                        )
                    # evict scaled by normalized gate weight; alternate the
                    # scale between ACT and DVE so neither engine saturates
                    t = tmp.tile([P, DH], F32, tag="evict")
                    if (m + n) % 2 == 0:
                        nc.scalar.mul(t, ps, ews[:, m, e : e + 1])
                    else:
                        nc.vector.tensor_scalar_mul(t, ps, ews[:, m, e : e + 1])
                    nc.vector.tensor_add(acc[m][:, nsl], acc[m][:, nsl], t)
                    if last:
                        noc = OC * 4 if m == MT - 1 else OC
                        for c in range(noc):
                            ocw2 = DH // noc
                            osl = slice(
                                n * DH + c * ocw2, n * DH + (c + 1) * ocw2
                            )
                            nc.sync.dma_start(
                                out_sh[m * P : (m + 1) * P, osl],
                                acc[m][:, osl],
                            )

    nc.compile()
    return nc


def kernel(**inputs) -> np.ndarray:
    global LAST_RESULT
    import ml_dtypes

    bf16 = ml_dtypes.bfloat16
    x = np.asarray(inputs["x"], dtype=np.float32).reshape(T, H)
    gw = np.ascontiguousarray(np.asarray(inputs["gate_w"], dtype=np.float32).astype(bf16))
    gb = np.ascontiguousarray(np.asarray(inputs["gate_b"], dtype=np.float32))
    ew = np.ascontiguousarray(np.asarray(inputs["expert_w"], dtype=np.float32).astype(bf16))
    eb = np.ascontiguousarray(np.asarray(inputs["expert_b"], dtype=np.float32))

    if "nc" not in _CACHE:
        _CACHE["nc"] = _build_moe_nc()
    nc = _CACHE["nc"]

    in_maps = [
        {
            "x_shT": np.ascontiguousarray(x[c * TL : (c + 1) * TL].T.astype(bf16)),
            "gate_w": gw,
            "gate_b": gb,
            "expert_w": ew,
            "expert_b": eb,
        }
        for c in range(N_CORES)
    ]
    res = run_bass_kernel_spmd(
        nc,
        in_maps,
        core_ids=list(range(N_CORES)),
        trace=bool(int(os.environ.get("MOE_TRACE", "0"))),
    )
    LAST_RESULT = res
    out = np.concatenate([res.results[c]["out_sh"] for c in range(N_CORES)], axis=0)
    return out.reshape(B, S, H)
